# revision 3
# baseline (speedup 1.0000x reference)
"""Trainium2 Bass kernel for the bilinear classifier problem.

Reference computation (single full-shape op):
    xW     = x @ W                     # [512, 512]
    scores = xW @ embedding_matrix.T   # [512, 20000]

Sharding (classifier / tensor parallel over num_class, per sharding hint):
  - embedding_matrix (and output scores) sharded over num_class across 8 cores
  - x, W replicated on every core
  - no collectives: each core writes its score shard, host concatenates

Per-core device kernel (TensorE convention: out[M,N] = lhsT[K,M]^T @ rhs[K,N]):
  GEMM1: xWT[w, b] = sum_i W[i,w] * xT[i,b]    lhsT=W [1024,512], rhs=xT [1024,512]
  GEMM2: out[b, c] = sum_w xWT[w,b] * ET[w,c]  lhsT=xWT [512,512], rhs=ET_shard [512,2560]

Host-side prep (layout only, no FLOPs of the reference op): transpose x and E,
zero-pad num_class 20000 -> 20480 so each core gets an even 2560 columns.
"""

import os

import numpy as np

B = 512
IMG_D = 1024
WORD_D = 512
NUM_CLASS = 20000
N_CORES = 8
C_PAD = 20480  # next multiple of 8*512
C_SHARD = C_PAD // N_CORES  # 2560
N_TILE = 512
N_CHUNKS = C_SHARD // N_TILE  # 5

_CACHE = {}


def _build_nc(mm_dtype: str):
    import concourse.mybir as mybir
    import concourse.tile as tile
    from concourse import bacc

    f32 = mybir.dt.float32
    f32r = mybir.dt.float32r
    bf16 = mybir.dt.bfloat16

    nc = bacc.Bacc(None, target_bir_lowering=False, debug=False)

    xT_ext = nc.declare_dram_parameter("xT", [IMG_D, B], f32, isOutput=False)
    W_ext = nc.declare_dram_parameter("W", [IMG_D, WORD_D], f32, isOutput=False)
    ET_ext = nc.declare_dram_parameter("ET", [WORD_D, C_SHARD], f32, isOutput=False)
    out_ext = nc.declare_dram_parameter("out", [B, C_SHARD], f32, isOutput=True)

    KO1 = IMG_D // 128  # 8 k-subtiles for GEMM1
    KO2 = WORD_D // 128  # 4 k-subtiles for GEMM2
    MW = WORD_D // 128  # 4 m-subtiles of xWT
    MB = B // 128  # 4 m-subtiles of scores

    def mm_ap(ap):
        if mm_dtype == "f32r":
            return ap.bitcast(f32r)
        return ap

    with tile.TileContext(nc) as tc:
        with (
            tc.tile_pool(name="const", bufs=1) as cpool,
            tc.tile_pool(name="work", bufs=3) as pool,
            tc.tile_pool(name="outp", bufs=3) as opool,
            tc.tile_pool(name="ps1", bufs=2, space="PSUM") as ps1,
            tc.tile_pool(name="ps2", bufs=4, space="PSUM") as ps2,
        ):
            # --- load W and xT as [128, ko, free] ---
            w_sb = cpool.tile([128, KO1, WORD_D], f32)
            nc.sync.dma_start(
                w_sb[:], W_ext[:].rearrange("(ko p) m -> p ko m", p=128)
            )
            x_sb = cpool.tile([128, KO1, B], f32)
            nc.sync.dma_start(
                x_sb[:], xT_ext[:].rearrange("(ko p) n -> p ko n", p=128)
            )

            if mm_dtype == "bf16":
                w_mm = cpool.tile([128, KO1, WORD_D], bf16)
                nc.vector.tensor_copy(out=w_mm[:], in_=w_sb[:])
                x_mm = cpool.tile([128, KO1, B], bf16)
                nc.vector.tensor_copy(out=x_mm[:], in_=x_sb[:])
            else:
                w_mm, x_mm = w_sb, x_sb

            # --- GEMM1: xWT [512(w), 512(b)] ---
            xwt_dt = bf16 if mm_dtype == "bf16" else f32
            xwt_sb = cpool.tile([128, MW, B], xwt_dt)
            for mw in range(MW):
                ps = ps1.tile([128, B], f32, tag="g1psum")
                for k in range(KO1):
                    nc.tensor.matmul(
                        ps[:],
                        lhsT=mm_ap(w_mm[:, k, mw * 128 : (mw + 1) * 128]),
                        rhs=mm_ap(x_mm[:, k, :]),
                        start=(k == 0),
                        stop=(k == KO1 - 1),
                    )
                nc.vector.tensor_copy(out=xwt_sb[:, mw, :], in_=ps[:])

            # --- GEMM2: out [512(b), 2560(c)] in chunks of 512 cols ---
            for n in range(N_CHUNKS):
                et_sb = pool.tile([128, KO2, N_TILE], f32, tag="et")
                nc.sync.dma_start(
                    et_sb[:],
                    ET_ext[:, n * N_TILE : (n + 1) * N_TILE].rearrange(
                        "(ko p) c -> p ko c", p=128
                    ),
                )
                if mm_dtype == "bf16":
                    et_mm = pool.tile([128, KO2, N_TILE], bf16, tag="etb")
                    nc.vector.tensor_copy(out=et_mm[:], in_=et_sb[:])
                else:
                    et_mm = et_sb
                for mb in range(MB):
                    ps = ps2.tile([128, N_TILE], f32, tag="g2psum")
                    for k in range(KO2):
                        nc.tensor.matmul(
                            ps[:],
                            lhsT=mm_ap(xwt_sb[:, k, mb * 128 : (mb + 1) * 128]),
                            rhs=mm_ap(et_mm[:, k, :]),
                            start=(k == 0),
                            stop=(k == KO2 - 1),
                        )
                    o_sb = opool.tile([128, N_TILE], f32, tag="osb")
                    nc.vector.tensor_copy(out=o_sb[:], in_=ps[:])
                    # outputs ride the scalar engine's HWDGE queue so they
                    # don't head-of-line block input loads on the sync queue
                    nc.scalar.dma_start(
                        out_ext[mb * 128 : (mb + 1) * 128, n * N_TILE : (n + 1) * N_TILE],
                        o_sb[:],
                    )

    nc.compile()
    return nc


def _get_nc():
    mm_dtype = os.environ.get("KERNEL_MM_DTYPE", "f32")
    key = ("nc", mm_dtype)
    if key not in _CACHE:
        _CACHE[key] = _build_nc(mm_dtype)
    return _CACHE[key]


def kernel(x: np.ndarray, embedding_matrix: np.ndarray, W: np.ndarray) -> np.ndarray:
    from concourse.bass_utils import run_bass_kernel_spmd

    trace = os.environ.get("KERNEL_TRACE", "0") == "1"

    xT = np.ascontiguousarray(np.asarray(x, dtype=np.float32).T)
    Wc = np.ascontiguousarray(np.asarray(W, dtype=np.float32))
    ET = np.zeros((WORD_D, C_PAD), dtype=np.float32)
    ET[:, :NUM_CLASS] = np.asarray(embedding_matrix, dtype=np.float32).T

    in_maps = [
        {
            "xT": xT,
            "W": Wc,
            "ET": np.ascontiguousarray(ET[:, c * C_SHARD : (c + 1) * C_SHARD]),
        }
        for c in range(N_CORES)
    ]

    nc = _get_nc()
    tmpdir = os.environ.get("KERNEL_TRACE_DIR") if trace else None
    if tmpdir:
        os.makedirs(tmpdir, exist_ok=True)
    res = run_bass_kernel_spmd(
        nc, in_maps, core_ids=list(range(N_CORES)), trace=trace, tmpdir=tmpdir
    )
    if trace:
        _CACHE["last_exec_time_ns"] = res.exec_time_ns
        _CACHE["last_trace"] = res.instructions_and_trace

    full = np.concatenate([res.results[c]["out"] for c in range(N_CORES)], axis=1)
    return np.ascontiguousarray(full[:, :NUM_CLASS])


# revision 5
# speedup vs baseline: 1.1923x; 1.1923x over previous
"""Trainium2 Bass kernel for the bilinear classifier problem.

Reference computation (single full-shape op):
    xW     = x @ W                     # [512, 512]
    scores = xW @ embedding_matrix.T   # [512, 20000]

Sharding (classifier / tensor parallel over num_class, per sharding hint):
  - embedding_matrix (and output scores) sharded over num_class across 8 cores
  - x, W replicated on every core
  - no collectives: each core writes its score shard, host concatenates

Per-core device kernel (TensorE convention: out[M,N] = lhsT[K,M]^T @ rhs[K,N]):
  GEMM1: xWT[w, b] = sum_i W[i,w] * xT[i,b]    lhsT=W [1024,512], rhs=xT [1024,512]
  GEMM2: out[b, c] = sum_w xWT[w,b] * ET[w,c]  lhsT=xWT [512,512], rhs=ET_shard [512,2560]

Matmuls run in bf16 (inputs cast f32->bf16 inside the input DMA via the
gpsimd software-DGE queue; fp32 accumulate in PSUM; f32 output).

Host-side prep (layout only): transpose x and E, zero-pad num_class
20000 -> 20480 so each core gets an even 2560 columns.
"""

import os

import numpy as np

B = 512
IMG_D = 1024
WORD_D = 512
NUM_CLASS = 20000
N_CORES = 8
C_PAD = 20480  # next multiple of 8*512
C_SHARD = C_PAD // N_CORES  # 2560
N_TILE = 512
N_CHUNKS = C_SHARD // N_TILE  # 5

_CACHE = {}


def _build_nc():
    import concourse.mybir as mybir
    import concourse.tile as tile
    from concourse import bacc

    f32 = mybir.dt.float32
    bf16 = mybir.dt.bfloat16

    nc = bacc.Bacc(None, target_bir_lowering=False, debug=False)

    xT_ext = nc.declare_dram_parameter("xT", [IMG_D, B], f32, isOutput=False)
    W_ext = nc.declare_dram_parameter("W", [IMG_D, WORD_D], f32, isOutput=False)
    ET_ext = nc.declare_dram_parameter("ET", [WORD_D, C_SHARD], f32, isOutput=False)
    out_ext = nc.declare_dram_parameter("out", [B, C_SHARD], f32, isOutput=True)

    KO1 = IMG_D // 128  # 8 k-subtiles for GEMM1
    KO2 = WORD_D // 128  # 4 k-subtiles for GEMM2
    MW = WORD_D // 128  # 4 m-subtiles of xWT
    MB = B // 128  # 4 m-subtiles of scores

    with tile.TileContext(nc) as tc:
        with (
            tc.tile_pool(name="const", bufs=1) as cpool,
            tc.tile_pool(name="outp", bufs=6) as opool,
            tc.tile_pool(name="ps1", bufs=2, space="PSUM") as ps1,
            tc.tile_pool(name="ps2", bufs=6, space="PSUM") as ps2,
        ):
            # --- inputs: gpsimd software-DGE casting DMA, f32 DRAM -> bf16 SBUF.
            # Each tensor gets a dedicated tile (no pool recycling) so input
            # streaming never stalls on buffer reuse.
            w_sb = cpool.tile([128, KO1, WORD_D], bf16, name="w_sb")
            nc.gpsimd.dma_start(
                w_sb[:], W_ext[:].rearrange("(ko p) m -> p ko m", p=128)
            )
            x_sb = cpool.tile([128, KO1, B], bf16, name="x_sb")
            nc.gpsimd.dma_start(
                x_sb[:], xT_ext[:].rearrange("(ko p) n -> p ko n", p=128)
            )
            et_sb = []
            for n in range(N_CHUNKS):
                t = cpool.tile([128, KO2, N_TILE], bf16, name=f"et{n}")
                nc.gpsimd.dma_start(
                    t[:],
                    ET_ext[:, n * N_TILE : (n + 1) * N_TILE].rearrange(
                        "(ko p) c -> p ko c", p=128
                    ),
                )
                et_sb.append(t)

            # --- GEMM1: xWT [512(w), 512(b)] ---
            xwt_sb = cpool.tile([128, MW, B], bf16, name="xwt")
            for mw in range(MW):
                ps = ps1.tile([128, B], f32, tag="g1psum")
                for k in range(KO1):
                    nc.tensor.matmul(
                        ps[:],
                        lhsT=w_sb[:, k, mw * 128 : (mw + 1) * 128],
                        rhs=x_sb[:, k, :],
                        start=(k == 0),
                        stop=(k == KO1 - 1),
                    )
                nc.vector.tensor_copy(out=xwt_sb[:, mw, :], in_=ps[:])

            # --- GEMM2: out [512(b), 2560(c)] in column chunks of 512 ---
            for n in range(N_CHUNKS):
                for mb in range(MB):
                    ps = ps2.tile([128, N_TILE], f32, tag="g2psum")
                    for k in range(KO2):
                        nc.tensor.matmul(
                            ps[:],
                            lhsT=xwt_sb[:, k, mb * 128 : (mb + 1) * 128],
                            rhs=et_sb[n][:, k, :],
                            start=(k == 0),
                            stop=(k == KO2 - 1),
                        )
                    o_sb = opool.tile([128, N_TILE], f32, tag="osb")
                    # split PSUM evictions across DVE and ACT so neither
                    # engine serializes the pipeline
                    if (n * MB + mb) % 2 == 0:
                        nc.vector.tensor_copy(out=o_sb[:], in_=ps[:])
                    else:
                        nc.scalar.copy(out=o_sb[:], in_=ps[:])
                    nc.sync.dma_start(
                        out_ext[mb * 128 : (mb + 1) * 128, n * N_TILE : (n + 1) * N_TILE],
                        o_sb[:],
                    )

    nc.compile()
    return nc


def _get_nc():
    if "nc" not in _CACHE:
        _CACHE["nc"] = _build_nc()
    return _CACHE["nc"]


def kernel(x: np.ndarray, embedding_matrix: np.ndarray, W: np.ndarray) -> np.ndarray:
    from concourse.bass_utils import run_bass_kernel_spmd

    trace = os.environ.get("KERNEL_TRACE", "0") == "1"

    xT = np.ascontiguousarray(np.asarray(x, dtype=np.float32).T)
    Wc = np.ascontiguousarray(np.asarray(W, dtype=np.float32))
    ET = np.zeros((WORD_D, C_PAD), dtype=np.float32)
    ET[:, :NUM_CLASS] = np.asarray(embedding_matrix, dtype=np.float32).T

    in_maps = [
        {
            "xT": xT,
            "W": Wc,
            "ET": np.ascontiguousarray(ET[:, c * C_SHARD : (c + 1) * C_SHARD]),
        }
        for c in range(N_CORES)
    ]

    nc = _get_nc()
    tmpdir = os.environ.get("KERNEL_TRACE_DIR") if trace else None
    if tmpdir:
        os.makedirs(tmpdir, exist_ok=True)
    res = run_bass_kernel_spmd(
        nc, in_maps, core_ids=list(range(N_CORES)), trace=trace, tmpdir=tmpdir
    )
    if trace:
        _CACHE["last_exec_time_ns"] = res.exec_time_ns
        _CACHE["last_trace"] = res.instructions_and_trace

    full = np.concatenate([res.results[c]["out"] for c in range(N_CORES)], axis=1)
    return np.ascontiguousarray(full[:, :NUM_CLASS])


# revision 6
# speedup vs baseline: 1.2151x; 1.0191x over previous
"""Trainium2 Bass kernel for the bilinear classifier problem.

Reference computation (single full-shape op):
    xW     = x @ W                     # [512, 512]
    scores = xW @ embedding_matrix.T   # [512, 20000]

Sharding (classifier / tensor parallel over num_class, per sharding hint):
  - embedding_matrix (and output scores) sharded over num_class across 8 cores
  - x, W replicated on every core
  - no collectives: each core writes its score shard, host concatenates

Per-core device kernel (TensorE convention: out[M,N] = lhsT[K,M]^T @ rhs[K,N]):
  GEMM1: xWT[w, b] = sum_i W[i,w] * xT[i,b]    lhsT=W [1024,512], rhs=xT [1024,512]
  GEMM2: out[b, c] = sum_w xWT[w,b] * ET[w,c]  lhsT=xWT [512,512], rhs=ET_shard [512,2560]

Matmuls run in bf16 (inputs cast f32->bf16 inside the input DMA via the
gpsimd software-DGE queue; fp32 accumulate in PSUM; f32 output).

Host-side prep (layout only, no reference FLOPs): inputs are pre-arranged
into the SBUF-blocked layout [128 partitions, ...] so every DMA descriptor
reads a long contiguous run per partition:
  wx  [128, (KO1)(WORD_D + B)]      W and xT k-subtile-blocked, merged
  ET  [128, N_CHUNKS, KO2, N_TILE]  E^T column-chunked + k-subtile-blocked
num_class is zero-padded 20000 -> 20480 so each core gets 2560 columns.
"""

import os

import numpy as np

B = 512
IMG_D = 1024
WORD_D = 512
NUM_CLASS = 20000
N_CORES = 8
C_PAD = 20480  # next multiple of 8*512
C_SHARD = C_PAD // N_CORES  # 2560
N_TILE = 512
N_CHUNKS = C_SHARD // N_TILE  # 5

KO1 = IMG_D // 128  # 8 k-subtiles for GEMM1
KO2 = WORD_D // 128  # 4 k-subtiles for GEMM2
MW = WORD_D // 128  # 4 m-subtiles of xWT
MB = B // 128  # 4 m-subtiles of scores

_CACHE = {}


def _build_nc():
    import concourse.mybir as mybir
    import concourse.tile as tile
    from concourse import bacc

    f32 = mybir.dt.float32
    bf16 = mybir.dt.bfloat16

    nc = bacc.Bacc(None, target_bir_lowering=False, debug=False)

    wx_ext = nc.declare_dram_parameter(
        "wx", [128, KO1 * (WORD_D + B)], f32, isOutput=False
    )
    ET_ext = nc.declare_dram_parameter(
        "ET", [128, N_CHUNKS, KO2, N_TILE], f32, isOutput=False
    )
    out_ext = nc.declare_dram_parameter("out", [B, C_SHARD], f32, isOutput=True)

    with tile.TileContext(nc) as tc:
        with (
            tc.tile_pool(name="const", bufs=1) as cpool,
            tc.tile_pool(name="outp", bufs=6) as opool,
            tc.tile_pool(name="ps1", bufs=2, space="PSUM") as ps1,
            tc.tile_pool(name="ps2", bufs=6, space="PSUM") as ps2,
        ):
            # --- inputs: gpsimd software-DGE casting DMA, f32 DRAM -> bf16
            # SBUF. Dedicated tiles (no recycling): streaming never stalls.
            wx_sb = cpool.tile([128, KO1 * (WORD_D + B)], bf16, name="wx_sb")
            nc.gpsimd.dma_start(wx_sb[:], wx_ext[:])
            w_sb = wx_sb.rearrange("p (ko m) -> p ko m", ko=KO1)[:, :, :WORD_D]
            x_sb = wx_sb.rearrange("p (ko m) -> p ko m", ko=KO1)[:, :, WORD_D:]

            et_sb = []
            for n in range(N_CHUNKS):
                t = cpool.tile([128, KO2, N_TILE], bf16, name=f"et{n}")
                nc.gpsimd.dma_start(t[:], ET_ext[:, n])
                et_sb.append(t)

            # --- GEMM1: xWT [512(w), 512(b)] ---
            xwt_sb = cpool.tile([128, MW, B], bf16, name="xwt")
            for mw in range(MW):
                ps = ps1.tile([128, B], f32, tag="g1psum")
                for k in range(KO1):
                    nc.tensor.matmul(
                        ps[:],
                        lhsT=w_sb[:, k, mw * 128 : (mw + 1) * 128],
                        rhs=x_sb[:, k, :],
                        start=(k == 0),
                        stop=(k == KO1 - 1),
                    )
                nc.vector.tensor_copy(out=xwt_sb[:, mw, :], in_=ps[:])

            # --- GEMM2: out [512(b), 2560(c)] in column chunks of 512 ---
            for n in range(N_CHUNKS):
                for mb in range(MB):
                    ps = ps2.tile([128, N_TILE], f32, tag="g2psum")
                    for k in range(KO2):
                        nc.tensor.matmul(
                            ps[:],
                            lhsT=xwt_sb[:, k, mb * 128 : (mb + 1) * 128],
                            rhs=et_sb[n][:, k, :],
                            start=(k == 0),
                            stop=(k == KO2 - 1),
                        )
                    o_sb = opool.tile([128, N_TILE], f32, tag="osb")
                    # split PSUM evictions across DVE and ACT so neither
                    # engine serializes the pipeline
                    if (n * MB + mb) % 2 == 0:
                        nc.vector.tensor_copy(out=o_sb[:], in_=ps[:])
                    else:
                        nc.scalar.copy(out=o_sb[:], in_=ps[:])
                    nc.sync.dma_start(
                        out_ext[mb * 128 : (mb + 1) * 128, n * N_TILE : (n + 1) * N_TILE],
                        o_sb[:],
                    )

    nc.compile()
    return nc


def _get_nc():
    if "nc" not in _CACHE:
        _CACHE["nc"] = _build_nc()
    return _CACHE["nc"]


def _prep_host(x, embedding_matrix, W):
    """Blocked input layouts. Pure layout transforms (transpose/pad/reshape)."""
    x = np.asarray(x, dtype=np.float32)
    W = np.asarray(W, dtype=np.float32)
    E = np.asarray(embedding_matrix, dtype=np.float32)

    # W [IMG_D, WORD_D] -> [128, KO1, WORD_D]
    W_blk = W.reshape(KO1, 128, WORD_D).transpose(1, 0, 2)
    # xT [IMG_D, B] -> [128, KO1, B]
    xT_blk = x.T.reshape(KO1, 128, B).transpose(1, 0, 2)
    # merged blob [128, KO1*(WORD_D+B)]
    wx = np.concatenate([W_blk, xT_blk], axis=2).reshape(128, KO1 * (WORD_D + B))
    wx = np.ascontiguousarray(wx)

    # ET [WORD_D, C_PAD] -> [128, N_CORES, N_CHUNKS, KO2, N_TILE]
    ET = np.zeros((WORD_D, C_PAD), dtype=np.float32)
    ET[:, :NUM_CLASS] = E.T
    # w = ko*128 + p ; c_global = core*C_SHARD + n*N_TILE + cc
    ET_blk = ET.reshape(KO2, 128, N_CORES, N_CHUNKS, N_TILE).transpose(1, 2, 3, 0, 4)
    ET_blk = np.ascontiguousarray(ET_blk)
    return wx, ET_blk


def kernel(x: np.ndarray, embedding_matrix: np.ndarray, W: np.ndarray) -> np.ndarray:
    from concourse.bass_utils import run_bass_kernel_spmd

    trace = os.environ.get("KERNEL_TRACE", "0") == "1"

    wx, ET_blk = _prep_host(x, embedding_matrix, W)

    in_maps = [
        {"wx": wx, "ET": np.ascontiguousarray(ET_blk[:, c])} for c in range(N_CORES)
    ]

    nc = _get_nc()
    tmpdir = os.environ.get("KERNEL_TRACE_DIR") if trace else None
    if tmpdir:
        os.makedirs(tmpdir, exist_ok=True)
    res = run_bass_kernel_spmd(
        nc, in_maps, core_ids=list(range(N_CORES)), trace=trace, tmpdir=tmpdir
    )
    if trace:
        _CACHE["last_exec_time_ns"] = res.exec_time_ns
        _CACHE["last_trace"] = res.instructions_and_trace

    full = np.concatenate([res.results[c]["out"] for c in range(N_CORES)], axis=1)
    return np.ascontiguousarray(full[:, :NUM_CLASS])
